# revision 12
# baseline (speedup 1.0000x reference)
"""Trainium2 Bass kernel for nn_BaselineSSM: h_t = A h_{t-1} + B x_t ; y_t = C h_t.

Full-input contract: kernel(x, A, B, C) takes the unsharded inputs
(x (16,2048,512), A/B/C (512,512) fp32) and returns (ys, hs), both
(16,2048,512) fp32, matching reference.reference().

Strategy (8 NeuronCores, data-parallel over batch, 2 rows/core):
  The seq-2048 recurrence is computed with a 2-level chunked (blocked) scan:
    L1: 128 chunks of K=16 steps, batched as R=256 rows (2 batch x 128 chunks)
        pass1 (zero-init, keeps chunk sums) + pass2 (true inits, emits h).
    L2: the 128 chunk-boundary states form another linear recurrence with
        transition A^16 -> chunked again (8 chunks of 16), and the 8
        second-level boundaries are solved sequentially with A^256.
  All state layouts keep hidden on partitions ([h%128 part, (h//128, r) free])
  so every matmul contracts over partitions with no data transposes between
  stages; x is transposed on entry and h/ys transposed on exit via TensorE.

  Matmul dtypes: everything float32r (fp32 with 12-bit mantissa, full PE rate)
  except the A^2..A^256 squaring chain which must stay fp32 (error analysis:
  power-chain rounding is the only catastrophic term; each f32r scan pass only
  compounds <=16 roundings thanks to the hierarchy => ~1e-3 total rel err).
"""

import numpy as np

import concourse.bass as bass
import concourse.tile as tile
from concourse import bacc, mybir
from concourse.bass_utils import run_bass_kernel_spmd
from concourse.masks import make_identity

F32 = mybir.dt.float32
F32R = mybir.dt.float32r

B_TOT, S, H = 16, 2048, 512
NCORES = 8
BL = B_TOT // NCORES          # 2 batch rows per core
K, CN = 16, 128               # level-1 chunk len / count (K*CN = S)
K2, C2 = 16, 8                # level-2 chunk len / count (K2*C2 = CN)
R = BL * CN                   # 256 scan rows (level 1)
HB = H // 128                 # 4 hidden blocks

# dtype config: scan/proj/bound matmuls in f32r, power chain in f32.
# (override before first kernel() call for experiments)
CONFIG = {"scan": F32R, "proj": F32R, "bound": F32R, "pow": F32}


def _mm4(nc, psum, lhsT_tiles, rhs_getter, n, col):
    """psum[:, ib*n:(ib+1)*n] = sum_jb lhsT_tiles[jb][:, ib*128:+128].T @ rhs(jb)"""
    for ib in range(HB):
        for jb in range(HB):
            nc.tensor.matmul(
                psum[:, ib * n:(ib + 1) * n],
                lhsT_tiles[jb][:, ib * col:(ib + 1) * col],
                rhs_getter(jb),
                start=(jb == 0),
                stop=(jb == HB - 1),
            )


def _transpose512(nc, ps_pool, out_pool, nat_tiles, out_dt, tag, ident):
    """nat_tiles: 4 x (128, 512) [rows, cols]; returns 4 x (128, 512) transposed."""
    out = []
    for jb in range(HB):
        ps = ps_pool.tile([128, H], F32, tag="trw")
        for ib in range(HB):
            nc.tensor.transpose(
                ps[:, ib * 128:(ib + 1) * 128],
                nat_tiles[ib][:, jb * 128:(jb + 1) * 128].bitcast(F32),
                ident,
            )
        t = out_pool.tile([128, H], out_dt, tag=f"{tag}{jb}")
        nc.any.tensor_copy(t[:], ps[:])
        out.append(t)
    return out


def build(cfg):
    sdt, pdt, bdt, wdt = cfg["scan"], cfg["proj"], cfg["bound"], cfg["pow"]
    assert sdt == pdt, "ys-proj reuses the scan state tile as rhs"

    nc = bacc.Bacc("TRN2", target_bir_lowering=False, debug=False,
                   num_devices=NCORES)
    x_d = nc.dram_tensor("x", [BL, S, H], F32, kind="ExternalInput")
    A_d = nc.dram_tensor("A", [H, H], F32, kind="ExternalInput")
    B_d = nc.dram_tensor("B", [H, H], F32, kind="ExternalInput")
    C_d = nc.dram_tensor("C", [H, H], F32, kind="ExternalInput")
    ys_d = nc.dram_tensor("ys", [BL, S, H], F32, kind="ExternalOutput")
    hs_d = nc.dram_tensor("hs", [BL, S, H], F32, kind="ExternalOutput")

    with tile.TileContext(nc, pool_alloc_mode="queue") as tc:
        _build_body(nc, tc, x_d, A_d, B_d, C_d, ys_d, hs_d, sdt, pdt, bdt, wdt)
    nc.finalize()
    return nc


def _build_body(nc, tc, x_d, A_d, B_d, C_d, ys_d, hs_d, sdt, pdt, bdt, wdt):
    from contextlib import ExitStack
    est = ExitStack()
    with est:
        wpool = est.enter_context(tc.tile_pool(name="weights", bufs=1))

        ident = wpool.tile([128, 128], F32, tag="ident")
        make_identity(nc, ident[:])

        # ---------- phase 0: weights + powers of A -------------------------
        with tc.tile_pool(name="ph0", bufs=2) as p0, \
             tc.tile_pool(name="ph0ps", bufs=2, space="PSUM") as p0ps:

            def load_nat(dram, tag):
                nat = []
                for ib in range(HB):
                    t = p0.tile([128, H], F32, tag=f"{tag}{ib}")
                    nc.sync.dma_start(t[:], dram[ib * 128:(ib + 1) * 128, :])
                    nat.append(t)
                return nat

            b_nat = load_nat(B_d, "bn")
            c_nat = load_nat(C_d, "cn")
            a_nat = load_nat(A_d, "an")
            b_T = _transpose512(nc, p0ps, wpool, b_nat, pdt, "bT", ident)
            c_T = _transpose512(nc, p0ps, wpool, c_nat, pdt, "cT", ident)
            a_Ts = _transpose512(nc, p0ps, wpool, a_nat, sdt, "aTs", ident)

            # squaring chain in wdt (fp32): maintain P natural + P transposed
            if wdt == F32:
                a_Tw = _transpose512(nc, p0ps, p0, a_nat, wdt, "aTw", ident)
            else:
                a_Tw = a_Ts
            p_nat, p_T = a_nat, a_Tw
            a16_Tb = a256_Tb = None
            for lvl in range(8):
                q_nat = []
                for ab in range(HB):
                    ps = p0ps.tile([128, H], F32, tag="sqps")
                    for cb in range(HB):
                        nc.tensor.matmul(
                            ps[:], p_T[cb][:, ab * 128:(ab + 1) * 128],
                            p_nat[cb][:], start=(cb == 0), stop=(cb == HB - 1))
                    t = p0.tile([128, H], wdt, tag=f"q{ab}")
                    nc.any.tensor_copy(t[:], ps[:])
                    q_nat.append(t)
                q_T = _transpose512(nc, p0ps, p0, q_nat, wdt, "qT", ident)
                p_nat, p_T = q_nat, q_T
                if lvl == 3:   # A^16
                    a16_Tb = []
                    for jb in range(HB):
                        t = wpool.tile([128, H], bdt, tag=f"a16T{jb}")
                        nc.any.tensor_copy(t[:], q_T[jb][:].bitcast(F32))
                        a16_Tb.append(t)
                if lvl == 7:   # A^256
                    a256_Tb = []
                    for jb in range(HB):
                        t = wpool.tile([128, H], bdt, tag=f"a256T{jb}")
                        nc.any.tensor_copy(t[:], q_T[jb][:].bitcast(F32))
                        a256_Tb.append(t)

        # ---------- phase 0b + U-proj + L1 pass1 (xT lifetime) --------------
        u_pool = est.enter_context(tc.tile_pool(name="U", bufs=1))
        Ubuf = u_pool.tile([128, K * HB * R], F32, tag="Ubuf")   # (k, hblk, r)
        sc_pool = est.enter_context(tc.tile_pool(name="scan", bufs=2))
        scan_ps = est.enter_context(
            tc.tile_pool(name="scanps", bufs=2, space="PSUM"))

        with tc.tile_pool(name="xT", bufs=1) as xT_pool:
            xT = [xT_pool.tile([128, BL * S], pdt, tag=f"xT{jb}",
                               name=f"xT{jb}") for jb in range(HB)]
            with tc.tile_pool(name="xin", bufs=3) as xin_pool, \
                 tc.tile_pool(name="xtps", bufs=2, space="PSUM") as xtps:
                for b in range(BL):
                    for g in range(S // 128):
                        xt = xin_pool.tile([128, H], F32, tag="xin")
                        nc.sync.dma_start(xt[:], x_d[b, g * 128:(g + 1) * 128, :])
                        ps = xtps.tile([128, H], F32, tag="trx")
                        for jb in range(HB):
                            nc.tensor.transpose(
                                ps[:, jb * 128:(jb + 1) * 128],
                                xt[:, jb * 128:(jb + 1) * 128], ident)
                        t0 = b * S + g * 128
                        for jb in range(HB):
                            nc.any.tensor_copy(
                                xT[jb][:, t0:t0 + 128],
                                ps[:, jb * 128:(jb + 1) * 128])

            with tc.tile_pool(name="ups", bufs=2, space="PSUM") as u_ps_ctx:

                def u_proj(k):
                    ps = u_ps_ctx.tile([128, HB * R], F32, tag="ups")
                    _mm4(nc, ps, b_T, lambda jb: xT[jb][:, k::K], R, 128)
                    nc.any.tensor_copy(
                        Ubuf[:, k * HB * R:(k + 1) * HB * R], ps[:])

                u_proj(0)
                u_proj(1)
                # L0 = U[:, slice 0] cast to scan dtype (first-step rhs)
                L0 = sc_pool.tile([128, HB * R], sdt, tag="L")
                nc.any.tensor_copy(L0[:], Ubuf[:, 0:HB * R])
                Lprev = L0
                E = sc_pool.tile([128, HB * R], F32, tag="E", bufs=1)
                for k in range(1, K):
                    if k + 1 < K:
                        u_proj(k + 1)
                    ps = scan_ps.tile([128, HB * R], F32, tag="sps")
                    _mm4(nc, ps, a_Ts,
                         lambda jb: Lprev[:, jb * R:(jb + 1) * R], R, 128)
                    out = (E if k == K - 1 else
                           sc_pool.tile([128, HB * R], sdt, tag="L"))
                    for ib in range(HB):
                        sl = slice(ib * R, (ib + 1) * R)
                        usl = slice(k * HB * R + ib * R,
                                    k * HB * R + (ib + 1) * R)
                        nc.vector.tensor_add(out[:, sl], ps[:, sl], Ubuf[:, usl])
                    Lprev = out

        # E viewed as (p, hblk, b, c2, k2)
        E4 = E[:].rearrange("p (h b c2 k2) -> p h b c2 k2",
                            h=HB, b=BL, c2=C2, k2=K2)

        # ---------- L2 / L3 boundary hierarchy -----------------------------
        with tc.tile_pool(name="bnd", bufs=2) as bnd, \
             tc.tile_pool(name="bndps", bufs=2, space="PSUM") as bndps:
            R2 = BL * C2
            # L2 pass1 over E-slices, init = E[..., k2=0]
            Hl0 = bnd.tile([128, HB * R2], bdt, tag="Hl0")
            Hl0v = Hl0[:].rearrange("p (h b c2) -> p h b c2", h=HB, b=BL, c2=C2)
            nc.any.tensor_copy(Hl0v, E4[:, :, :, :, 0])
            Hlprev = Hl0
            E3 = bnd.tile([128, HB * R2], F32, tag="E3")  # level-2 sums
            for k2 in range(1, K2):
                ps = bndps.tile([128, HB * R2], F32, tag="bps")
                _mm4(nc, ps, a16_Tb,
                     lambda jb: Hlprev[:, jb * R2:(jb + 1) * R2], R2, 128)
                out = E3 if k2 == K2 - 1 else bnd.tile(
                    [128, HB * R2], bdt, tag="Hl")
                nc.vector.tensor_add(
                    out[:].rearrange("p (h b c2) -> p h b c2",
                                     h=HB, b=BL, c2=C2),
                    ps[:].rearrange("p (h b c2) -> p h b c2",
                                    h=HB, b=BL, c2=C2),
                    E4[:, :, :, :, k2])
                Hlprev = out
            E3v = E3[:].rearrange("p (h b c2) -> p h b c2", h=HB, b=BL, c2=C2)

            # L3: 8 sequential rounds with A^256; INIT2[c2] = state before c2
            zeros = sc_pool.tile([128, HB * R], F32, tag="zeros", bufs=1)
            nc.any.memset(zeros[:], 0.0)
            INIT2 = bnd.tile([128, HB * R2], bdt, tag="INIT2")
            INIT2v = INIT2[:].rearrange("p (h b c2) -> p h b c2",
                                        h=HB, b=BL, c2=C2)
            nc.any.tensor_copy(INIT2[:], zeros[:, 0:HB * R2])
            hb_t = bnd.tile([128, HB * BL], bdt, tag="Hb")
            nc.any.tensor_copy(
                hb_t[:].rearrange("p (h b) -> p h b", h=HB, b=BL),
                E3v[:, :, :, 0])
            nc.any.tensor_copy(INIT2v[:, :, :, 1], hb_t[:].rearrange(
                "p (h b) -> p h b", h=HB, b=BL))
            for c2 in range(1, C2):
                ps = bndps.tile([128, HB * BL], F32, tag="b3ps")
                _mm4(nc, ps, a256_Tb,
                     lambda jb: hb_t[:, jb * BL:(jb + 1) * BL], BL, 128)
                nxt = bnd.tile([128, HB * BL], bdt, tag="Hb")
                nc.vector.tensor_add(
                    nxt[:].rearrange("p (h b) -> p h b", h=HB, b=BL),
                    ps[:].rearrange("p (h b) -> p h b", h=HB, b=BL),
                    E3v[:, :, :, c2])
                hb_t = nxt
                if c2 + 1 < C2:
                    nc.any.tensor_copy(
                        INIT2v[:, :, :, c2 + 1],
                        hb_t[:].rearrange("p (h b) -> p h b", h=HB, b=BL))

            # L2 pass2: true level-1 inits -> INIT
            INIT = sc_pool.tile([128, HB * R], sdt, tag="INIT", bufs=1)
            INITv = INIT[:].rearrange("p (h b c2 k2) -> p h b c2 k2",
                                      h=HB, b=BL, c2=C2, k2=K2)
            # zero the whole tile; pass-2 writes fill every column except
            # (c2=0, k2=0), which must stay zero (chunk 0 has zero init)
            nc.any.tensor_copy(INIT[:], zeros[:])
            Hlprev = INIT2
            for k2 in range(K2):
                ps = bndps.tile([128, HB * R2], F32, tag="bps")
                _mm4(nc, ps, a16_Tb,
                     lambda jb: Hlprev[:, jb * R2:(jb + 1) * R2], R2, 128)
                out = bnd.tile([128, HB * R2], bdt, tag="Hl2")
                outv = out[:].rearrange("p (h b c2) -> p h b c2",
                                        h=HB, b=BL, c2=C2)
                nc.vector.tensor_add(
                    outv,
                    ps[:].rearrange("p (h b c2) -> p h b c2",
                                    h=HB, b=BL, c2=C2),
                    E4[:, :, :, :, k2])
                if k2 + 1 < K2:
                    nc.any.tensor_copy(INITv[:, :, :, :, k2 + 1], outv)
                else:
                    nc.any.tensor_copy(INITv[:, :, :, 1:, 0],
                                       outv[:, :, :, 0:C2 - 1])
                Hlprev = out

        # ---------- L1 pass2 + ys projection + outputs ---------------------
        with tc.tile_pool(name="out", bufs=4) as op, \
             tc.tile_pool(name="outps", bufs=2, space="PSUM") as ops, \
             tc.tile_pool(name="ypsp", bufs=1, space="PSUM") as ypsp:
            hprev = INIT
            for k in range(K):
                ps = scan_ps.tile([128, HB * R], F32, tag="sps")
                _mm4(nc, ps, a_Ts,
                     lambda jb: hprev[:, jb * R:(jb + 1) * R], R, 128)
                h = sc_pool.tile([128, HB * R], sdt, tag="h")
                for ib in range(HB):
                    sl = slice(ib * R, (ib + 1) * R)
                    usl = slice(k * HB * R + ib * R, k * HB * R + (ib + 1) * R)
                    nc.vector.tensor_add(h[:, sl], ps[:, sl], Ubuf[:, usl])
                hprev = h

                # ys = C @ h
                psy = ypsp.tile([128, HB * R], F32, tag="yps")
                _mm4(nc, psy, c_T, lambda hb: h[:, hb * R:(hb + 1) * R], R, 128)
                ysT = op.tile([128, HB * R], F32, tag="ysT")
                nc.any.tensor_copy(ysT[:], psy[:])

                # transpose h and ysT back to token-major and DMA out
                for src, dram in ((h, hs_d), (ysT, ys_d)):
                    for b in range(BL):
                        pst = ops.tile([128, H], F32, tag="trps")
                        for hb in range(HB):
                            nc.tensor.transpose(
                                pst[:, hb * 128:(hb + 1) * 128],
                                src[:, hb * R + b * CN: hb * R + (b + 1) * CN]
                                .bitcast(F32),
                                ident)
                        stg = op.tile([128, H], F32, tag="stg")
                        nc.any.tensor_copy(stg[:], pst[:])
                        nc.sync.dma_start(dram[b, k::K, :], stg[:])


_nc_cache = {}


def _get_nc():
    key = tuple(sorted((k, v.value) for k, v in CONFIG.items()))
    if key not in _nc_cache:
        _nc_cache[key] = build(CONFIG)
    return _nc_cache[key]


def kernel(x, A, B, C):
    nc = _get_nc()
    x = np.ascontiguousarray(np.asarray(x, np.float32))
    A = np.ascontiguousarray(np.asarray(A, np.float32))
    B = np.ascontiguousarray(np.asarray(B, np.float32))
    C = np.ascontiguousarray(np.asarray(C, np.float32))
    in_maps = [
        {"x": x[i * BL:(i + 1) * BL], "A": A, "B": B, "C": C}
        for i in range(NCORES)
    ]
    res = run_bass_kernel_spmd(nc, in_maps, list(range(NCORES)))
    ys = np.concatenate([res.results[i]["ys"] for i in range(NCORES)], axis=0)
    hs = np.concatenate([res.results[i]["hs"] for i in range(NCORES)], axis=0)
    return ys, hs
